# revision 13
# baseline (speedup 1.0000x reference)
"""MLA-style attention (shared latent KV head, attention sink, partial RoPE,
low-rank Q and grouped low-rank output projection) on 8 TRN2 NeuronCores.

Sharding: 64 query heads split 8 per core (tensor parallel on wq_b rows /
wo_a groups); latent KV path replicated; final wo_b matmul computed as
per-core partial products (each core owns one OLR group / one 1024-col slice
of wo_b) summed on the host.

All weights / activations are pre-laid-out on the host into the exact
[partition, ...] tile shapes the kernel wants, so every DMA is a contiguous
copy and the device never transposes anything except through the PE array
(qr -> qrT, q -> qT, p -> pT, o -> oT, kv -> kvT).
"""

import numpy as np
import ml_dtypes

import concourse.bass as bass
import concourse.mybir as mybir
import concourse.tile as tile
from concourse import bacc
from concourse.bass_utils import run_bass_kernel_spmd
from concourse.masks import make_identity, make_causal_mask

BF16 = mybir.dt.bfloat16
F32 = mybir.dt.float32
AX = mybir.AxisListType
ALU = mybir.AluOpType
ACTF = mybir.ActivationFunctionType

NPBF16 = ml_dtypes.bfloat16

# problem dims (hardcoded; kernel.py must be self-contained)
D, NH, HD, RD, QLR, OLR, OG = 4096, 64, 512, 64, 1024, 1024, 8
S = 1024
NCORES = 8
HPC = NH // NCORES  # query heads per core
EPS = 1e-6
P = 128


class Cfg:
    """Dimensions, parameterized so a shrunken config can run in CoreSim."""

    def __init__(self, s=S, d=D, qlr=QLR, hpc=HPC, olr=OLR, outd=D):
        assert s % P == 0 and d % P == 0 and qlr % 512 == 0 and olr % 512 == 0
        assert outd % 512 == 0
        self.s, self.d, self.qlr, self.hpc, self.olr, self.outd = (
            s, d, qlr, hpc, olr, outd)
        self.sc = s // P        # seq tiles
        self.dc = d // P        # model-dim chunks (contraction for qr/kv)
        self.qc = qlr // P      # q_lora chunks
        self.hc = HD // P       # head-dim chunks (4)
        self.f = hpc * HD       # per-core attention output feature dim
        self.fc = self.f // P   # feature chunks for wo_a contraction
        self.oc = olr // P      # olr chunks (contraction for wo_b)
        self.nc_out = outd // 512  # output D chunks


def _rope_inplace(nc, pool, dst, cos_ap, sin_ap, inverse):
    """Partial RoPE on dst[:, HD-RD:HD] in place. dst is [128, HD] bf16,
    cos/sin are [128, RD//2] f32 for this seq tile."""
    tail = dst[:, HD - RD:HD].rearrange("p (a two) -> p a two", two=2)
    x1 = tail[:, :, 0]
    x2 = tail[:, :, 1]
    t1 = pool.tile([P, RD // 2], F32, tag="rope1")
    t2 = pool.tile([P, RD // 2], F32, tag="rope2")
    t3 = pool.tile([P, RD // 2], F32, tag="rope3")
    t4 = pool.tile([P, RD // 2], F32, tag="rope4")
    nc.vector.tensor_mul(t1[:], x1, cos_ap)
    nc.vector.tensor_mul(t2[:], x2, sin_ap)
    nc.vector.tensor_mul(t3[:], x1, sin_ap)
    nc.vector.tensor_mul(t4[:], x2, cos_ap)
    if not inverse:
        # o1 = x1 c - x2 s ; o2 = x1 s + x2 c
        nc.vector.tensor_sub(x1, t1[:], t2[:])
        nc.vector.tensor_add(x2, t3[:], t4[:])
    else:
        # o1 = x1 c + x2 s ; o2 = x2 c - x1 s
        nc.vector.tensor_add(x1, t1[:], t2[:])
        nc.vector.tensor_sub(x2, t4[:], t3[:])


def build_program(cfg: Cfg, debug=False, reps=1):
    """Trace + schedule + compile the per-core program. Returns nc.
    reps>1 repeats the whole body (for steady-state timing)."""
    nc = bacc.Bacc("TRN2", debug=False, num_devices=NCORES)

    # ---- DRAM I/O (host supplies pre-tiled layouts) ----
    shard_a = (cfg.sc == NCORES)
    if shard_a:
        xt_d = nc.dram_tensor("xtm", [P, cfg.dc, P], BF16,
                              kind="ExternalInput").ap()
        cosm_d = nc.dram_tensor("cosm", [P, RD // 2], F32,
                                kind="ExternalInput").ap()
        sinm_d = nc.dram_tensor("sinm", [P, RD // 2], F32,
                                kind="ExternalInput").ap()
    else:
        xt_d = nc.dram_tensor("xt", [cfg.sc, P, cfg.dc, P], BF16,
                              kind="ExternalInput").ap()
        cosm_d = sinm_d = None
    wqa_d = nc.dram_tensor("wqa", [P, cfg.dc, cfg.qlr], BF16,
                           kind="ExternalInput").ap()
    wkv_d = nc.dram_tensor("wkv", [P, cfg.dc, HD], BF16,
                           kind="ExternalInput").ap()
    wqb_d = nc.dram_tensor("wqb", [P, cfg.qc, cfg.hpc * HD], BF16,
                           kind="ExternalInput").ap()
    woa_d = nc.dram_tensor("woa", [P, cfg.fc, cfg.olr], BF16,
                           kind="ExternalInput").ap()
    wob_d = nc.dram_tensor("wob", [P, cfg.oc, cfg.outd], BF16,
                           kind="ExternalInput").ap()
    cos_d = nc.dram_tensor("coss", [P, cfg.sc, RD // 2], F32,
                           kind="ExternalInput").ap()
    sin_d = nc.dram_tensor("sins", [P, cfg.sc, RD // 2], F32,
                           kind="ExternalInput").ap()
    kvw_d = nc.dram_tensor("kvw", [P, HD], BF16, kind="ExternalInput").ap()
    sink_d = nc.dram_tensor("sink", [P, cfg.hpc], F32,
                            kind="ExternalInput").ap()
    nsink_d = nc.dram_tensor("nsink", [P, cfg.hpc], F32,
                             kind="ExternalInput").ap()
    out_d = nc.dram_tensor("out", [cfg.sc, P, cfg.outd], F32,
                           kind="ExternalOutput").ap()
    dbg = {}
    if debug:
        dbg["qrt"] = nc.dram_tensor("dbg_qrt", [P, cfg.qc, cfg.s], BF16,
                                    kind="ExternalOutput").ap()
        dbg["kv"] = nc.dram_tensor("dbg_kv", [P, cfg.sc, HD], BF16,
                                   kind="ExternalOutput").ap()
        dbg["qt0"] = nc.dram_tensor("dbg_qt0", [P, cfg.hc, cfg.s], BF16,
                                    kind="ExternalOutput").ap()
        dbg["ot"] = nc.dram_tensor("dbg_ot", [P, cfg.fc, cfg.s], BF16,
                                   kind="ExternalOutput").ap()
        dbg["ogt"] = nc.dram_tensor("dbg_ogt", [P, cfg.oc, cfg.s], BF16,
                                    kind="ExternalOutput").ap()

    with tile.TileContext(nc) as tc:
        for _ in range(reps):
            _body(nc, tc, cfg, xt_d, wqa_d, wkv_d, wqb_d, woa_d, wob_d,
                  cos_d, sin_d, kvw_d, sink_d, nsink_d, out_d, dbg,
                  shard_a=shard_a, cosm_d=cosm_d, sinm_d=sinm_d)

    nc.compile()
    return nc


def _body(nc, tc, cfg, xt_d, wqa_d, wkv_d, wqb_d, woa_d, wob_d,
          cos_d, sin_d, kvw_d, sink_d, nsink_d, out_d, dbg=None,
          shard_a=False, cosm_d=None, sinm_d=None):
    sc, dc, qc, hc = cfg.sc, cfg.dc, cfg.qc, cfg.hc

    with tc.tile_pool(name="persist", bufs=1) as pp:
        ident = pp.tile([P, P], BF16)
        make_identity(nc, ident[:])
        cmask = pp.tile([P, P], F32)
        make_causal_mask(nc, cmask[:], mask_val=-1e10)
        kvw_sb = pp.tile([P, HD], BF16)
        nc.sync.dma_start(kvw_sb[:], kvw_d)
        sink_sb = pp.tile([P, cfg.hpc], F32)
        nc.sync.dma_start(sink_sb[:], sink_d)
        nsink_sb = pp.tile([P, cfg.hpc], F32)
        nc.sync.dma_start(nsink_sb[:], nsink_d)
        cos_sb = pp.tile([P, sc, RD // 2], F32)
        nc.sync.dma_start(cos_sb[:], cos_d)
        sin_sb = pp.tile([P, sc, RD // 2], F32)
        nc.sync.dma_start(sin_sb[:], sin_d)
        if shard_a:
            cosm_sb = pp.tile([P, RD // 2], F32)
            nc.sync.dma_start(cosm_sb[:], cosm_d)
            sinm_sb = pp.tile([P, RD // 2], F32)
            nc.sync.dma_start(sinm_sb[:], sinm_d)
        kv_sb = pp.tile([P, sc, HD], BF16)      # latent KV, [s-in-tile, tile, hd]
        kvT_sb = pp.tile([P, hc, cfg.s], BF16)  # latent KV transposed
        eps_sb = pp.tile([P, 2], F32)           # [:,0]=EPS, [:,1]=-ln(HD)/2
        nc.gpsimd.memset(eps_sb[:, 0:1], float(EPS))
        nc.gpsimd.memset(eps_sb[:, 1:2], float(-0.5 * np.log(HD)))

        with tc.tile_pool(name="qrt", bufs=1) as qrtp:
            qrT_sb = qrtp.tile([P, qc, cfg.s], BF16)

            # ================= stage A: qr + kv =================
            with tc.tile_pool(name="stA", bufs=1) as pa, \
                 tc.tile_pool(name="stAw", bufs=2) as paw, \
                 tc.tile_pool(name="psA", bufs=1, space="PSUM") as psa:
                # weights, chunked so compute can start early
                wqa_sb = pa.tile([P, dc, cfg.qlr], BF16)
                nsp = min(8, dc)
                for g in range(nsp):
                    gsz = dc // nsp
                    nc.sync.dma_start(wqa_sb[:, g * gsz:(g + 1) * gsz, :],
                                      wqa_d[:, g * gsz:(g + 1) * gsz, :])
                wkv_sb = pa.tile([P, dc, HD], BF16)
                nsp = min(4, dc)
                for g in range(nsp):
                    gsz = dc // nsp
                    nc.sync.dma_start(wkv_sb[:, g * gsz:(g + 1) * gsz, :],
                                      wkv_d[:, g * gsz:(g + 1) * gsz, :])

                for i in range([sc, 1][shard_a]):
                    xt_i = paw.tile([P, dc, P], BF16, tag="xt")
                    nc.sync.dma_start(xt_i[:], xt_d if shard_a else xt_d[i])
                    qr_ps = psa.tile([P, cfg.qlr], F32, tag="qr", bufs=2)
                    kv_ps = psa.tile([P, HD], F32, tag="kv", bufs=2)
                    for k in range(dc):
                        st, sp = k == 0, k == dc - 1
                        for n2 in range(cfg.qlr // 512):
                            nc.tensor.matmul(
                                qr_ps[:, n2 * 512:(n2 + 1) * 512],
                                xt_i[:, k, :],
                                wqa_sb[:, k, n2 * 512:(n2 + 1) * 512],
                                start=st, stop=sp)
                        nc.tensor.matmul(kv_ps[:], xt_i[:, k, :],
                                         wkv_sb[:, k, :], start=st, stop=sp)

                    # --- qr epilogue: cast, rmsnorm, transpose ---
                    qr_sb = paw.tile([P, cfg.qlr], BF16, tag="qr_sb")
                    nc.any.tensor_copy(qr_sb[:], qr_ps[:])
                    sq = paw.tile([P, cfg.qlr], F32, tag="sq")
                    ssq = paw.tile([P, 1], F32, tag="ssq")
                    nc.scalar.activation(sq[:], qr_sb[:], ACTF.Square,
                                         accum_out=ssq[:])
                    rt = paw.tile([P, 1], F32, tag="rt")
                    nc.scalar.activation(rt[:], ssq[:], ACTF.Ln,
                                         bias=eps_sb[:, 0:1],
                                         scale=1.0 / cfg.qlr)
                    rinv = paw.tile([P, 1], F32, tag="rinv")
                    nc.scalar.activation(rinv[:], rt[:], ACTF.Exp, scale=-0.5)
                    qrn = paw.tile([P, cfg.qlr], BF16, tag="qrn")
                    nc.scalar.mul(qrn[:], qr_sb[:], rinv[:])
                    if shard_a:
                        qrT_loc = paw.tile([P, qc, P], BF16, tag="qrT_loc",
                                           bufs=1)
                    for g in range((qc + 3) // 4):
                        jn = min(4, qc - g * 4)
                        tp = psa.tile([P, 512], BF16, tag="t", bufs=2)
                        for c4 in range(jn):
                            c = g * 4 + c4
                            nc.tensor.transpose(
                                tp[:, c4 * P:(c4 + 1) * P],
                                qrn[:, c * P:(c + 1) * P], ident[:])
                        dst = (qrT_loc[:, g * 4:g * 4 + jn, :] if shard_a else
                               qrT_sb[:, g * 4:g * 4 + jn, i * P:(i + 1) * P])
                        nc.any.tensor_copy(
                            dst,
                            tp[:, :jn * P].rearrange("p (c s) -> p c s", c=jn))

                    # --- kv epilogue: cast, rmsnorm, weight, rope, transpose ---
                    kvt = paw.tile([P, HD], BF16, tag="kvt")
                    nc.any.tensor_copy(kvt[:], kv_ps[:])
                    sqk = paw.tile([P, HD], F32, tag="sqk")
                    ssqk = paw.tile([P, 1], F32, tag="ssqk")
                    nc.scalar.activation(sqk[:], kvt[:], ACTF.Square,
                                         accum_out=ssqk[:])
                    rtk = paw.tile([P, 1], F32, tag="rtk")
                    nc.scalar.activation(rtk[:], ssqk[:], ACTF.Ln,
                                         bias=eps_sb[:, 0:1], scale=1.0 / HD)
                    rinvk = paw.tile([P, 1], F32, tag="rinvk")
                    nc.scalar.activation(rinvk[:], rtk[:], ACTF.Exp,
                                         scale=-0.5)
                    if shard_a:
                        kv_dst = paw.tile([P, HD], BF16, tag="kv_loc", bufs=1)
                        kv_dst = kv_dst[:]
                        cos_i, sin_i = cosm_sb[:], sinm_sb[:]
                    else:
                        kv_dst = kv_sb[:, i, :]
                        cos_i, sin_i = cos_sb[:, i, :], sin_sb[:, i, :]
                    nc.scalar.mul(kv_dst, kvt[:], rinvk[:])
                    nc.vector.tensor_mul(kv_dst, kv_dst, kvw_sb[:])
                    _rope_inplace(nc, paw, kv_dst, cos_i, sin_i, False)
                    tpk = psa.tile([P, 512], BF16, tag="t", bufs=2)
                    for c in range(hc):
                        nc.tensor.transpose(tpk[:, c * P:(c + 1) * P],
                                            kv_dst[:, c * P:(c + 1) * P],
                                            ident[:])
                    if shard_a:
                        kvT_loc = paw.tile([P, hc, P], BF16, tag="kvT_loc",
                                           bufs=1)
                        nc.any.tensor_copy(
                            kvT_loc[:],
                            tpk[:].rearrange("p (c s) -> p c s", c=hc))
                        # pack local results into DRAM and all-gather
                        gw = qc * P + HD + hc * P      # 2048 for full cfg
                        with tc.tile_pool(name="ccdram", bufs=1,
                                          space="DRAM") as ccd:
                            gin = ccd.tile([P, gw], BF16)
                            gout = ccd.tile([NCORES, P, gw], BF16)
                            nc.sync.dma_start(gin[:, 0:qc * P],
                                              qrT_loc[:].rearrange(
                                                  "p c s -> p (c s)"))
                            nc.sync.dma_start(
                                gin[:, qc * P:qc * P + HD], kv_dst)
                            nc.sync.dma_start(gin[:, qc * P + HD:gw],
                                              kvT_loc[:].rearrange(
                                                  "p c s -> p (c s)"))
                            nc.gpsimd.collective_compute(
                                "AllGather", ALU.bypass,
                                replica_groups=[list(range(NCORES))],
                                ins=[gin[:]], outs=[gout[:]])
                            for j in range(NCORES):
                                nc.sync.dma_start(
                                    qrT_sb[:, :, j * P:(j + 1) * P],
                                    gout[j, :, 0:qc * P].rearrange(
                                        "p (c s) -> p c s", c=qc))
                                nc.sync.dma_start(
                                    kv_sb[:, j, :],
                                    gout[j, :, qc * P:qc * P + HD])
                                nc.sync.dma_start(
                                    kvT_sb[:, :, j * P:(j + 1) * P],
                                    gout[j, :, qc * P + HD:gw].rearrange(
                                        "p (c s) -> p c s", c=hc))
                    else:
                        nc.any.tensor_copy(
                            kvT_sb[:, :, i * P:(i + 1) * P],
                            tpk[:].rearrange("p (c s) -> p c s", c=hc))

            if dbg:
                nc.sync.dma_start(dbg["qrt"], qrT_sb[:])
                nc.sync.dma_start(dbg["kv"], kv_sb[:])

            # ================= stage BC: per-head q proj + attention ========
            with tc.tile_pool(name="ot", bufs=1) as otp:
                oT_sb = otp.tile([P, cfg.fc, cfg.s], BF16)

                with tc.tile_pool(name="stBC", bufs=1) as pb, \
                     tc.tile_pool(name="stBCw", bufs=2) as pbw, \
                     tc.tile_pool(name="psQ", bufs=1, space="PSUM") as psq, \
                     tc.tile_pool(name="psS", bufs=1, space="PSUM") as pss, \
                     tc.tile_pool(name="psT", bufs=1, space="PSUM") as pst, \
                     tc.tile_pool(name="psO", bufs=1, space="PSUM") as pso:
                    wqb_sb = pb.tile([P, qc, cfg.hpc * HD], BF16)
                    for g in range(qc):
                        nc.sync.dma_start(wqb_sb[:, g:g + 1, :],
                                          wqb_d[:, g:g + 1, :])

                    for h in range(cfg.hpc):
                        qT_sb = pbw.tile([P, hc, cfg.s], BF16, tag="qT")
                        # ---- q projection + per-head RMS norm + rope ----
                        q8 = pbw.tile([P, sc, HD], BF16, tag="q8", bufs=1)
                        ssq8 = pbw.tile([P, sc], F32, tag="ssq8")
                        for i in range(sc):
                            q_ps = psq.tile([P, HD], F32, tag="q", bufs=1)
                            for c in range(qc):
                                nc.tensor.matmul(
                                    q_ps[:],
                                    qrT_sb[:, c, i * P:(i + 1) * P],
                                    wqb_sb[:, c, h * HD:(h + 1) * HD],
                                    start=(c == 0), stop=(c == qc - 1))
                            nc.any.tensor_copy(q8[:, i, :], q_ps[:])
                            sqq = pbw.tile([P, HD], F32, tag="sqq")
                            nc.scalar.activation(sqq[:], q8[:, i, :],
                                                 ACTF.Square,
                                                 accum_out=ssq8[:, i:i + 1])
                        # rsqrt(ms+eps)/sqrt(HD) = exp(-0.5*ln(ssq/HD+eps)
                        #                              - 0.5*ln(HD))
                        rt8 = pbw.tile([P, sc], F32, tag="rt8")
                        nc.scalar.activation(rt8[:], ssq8[:], ACTF.Ln,
                                             bias=eps_sb[:, 0:1],
                                             scale=1.0 / HD)
                        rinv8 = pbw.tile([P, sc], F32, tag="rinv8")
                        nc.scalar.activation(rinv8[:], rt8[:], ACTF.Exp,
                                             scale=-0.5,
                                             bias=eps_sb[:, 1:2])
                        for i in range(sc):
                            nc.scalar.mul(q8[:, i, :], q8[:, i, :],
                                          rinv8[:, i:i + 1])
                            _rope_inplace(nc, pbw, q8[:, i, :],
                                          cos_sb[:, i, :], sin_sb[:, i, :],
                                          False)
                            tpq = pst.tile([P, 512], BF16, tag="t", bufs=2)
                            for c in range(hc):
                                nc.tensor.transpose(
                                    tpq[:, c * P:(c + 1) * P],
                                    q8[:, i, c * P:(c + 1) * P], ident[:])
                            nc.any.tensor_copy(
                                qT_sb[:, :, i * P:(i + 1) * P],
                                tpq[:].rearrange("p (c s) -> p c s", c=hc))

                        if dbg and h == 0:
                            nc.sync.dma_start(dbg["qt0"], qT_sb[:])

                        # ---- attention for head h ----
                        for i in range(sc):
                            w_all = (i + 1) * P
                            nch = (w_all + 511) // 512
                            s_ps = []
                            for ci in range(nch):
                                wci = min(512, w_all - ci * 512)
                                s_ps.append((pss.tile([P, 512], F32, tag="s",
                                                      bufs=3, name="s_ps"),
                                             wci))
                            for k in range(hc):
                                for ci in range(nch):
                                    tile_ps, wci = s_ps[ci]
                                    nc.tensor.matmul(
                                        tile_ps[:, :wci],
                                        qT_sb[:, k, i * P:(i + 1) * P],
                                        kvT_sb[:, k, ci * 512:ci * 512 + wci],
                                        start=(k == 0), stop=(k == hc - 1))
                            # causal mask on the diagonal block
                            dps, dw = s_ps[-1]
                            dcol = (w_all - P) - (nch - 1) * 512
                            nc.vector.tensor_add(dps[:, dcol:dcol + P],
                                                 dps[:, dcol:dcol + P],
                                                 cmask[:])
                            # negated row max (incl. sink)
                            nmt = pbw.tile([P, 3], F32, tag="nmt")
                            for ci in range(nch):
                                tile_ps, wci = s_ps[ci]
                                nc.vector.reduce_max(nmt[:, ci:ci + 1],
                                                     tile_ps[:, :wci],
                                                     axis=AX.X, negate=True)
                            nm = pbw.tile([P, 1], F32, tag="nm")
                            if nch == 1:
                                nc.vector.tensor_tensor(
                                    nm[:], nmt[:, 0:1], nsink_sb[:, h:h + 1],
                                    op=ALU.min)
                            else:
                                nc.vector.tensor_tensor(
                                    nm[:], nmt[:, 0:1], nmt[:, 1:2],
                                    op=ALU.min)
                                nc.vector.tensor_tensor(
                                    nm[:], nm[:], nsink_sb[:, h:h + 1],
                                    op=ALU.min)
                            # exp + row sums
                            p_sb = pbw.tile([P, cfg.s], BF16, tag="p")
                            l0 = pbw.tile([P, 4], F32, tag="l0")
                            for ci in range(nch):
                                tile_ps, wci = s_ps[ci]
                                nc.scalar.activation(
                                    p_sb[:, ci * 512:ci * 512 + wci],
                                    tile_ps[:, :wci], ACTF.Exp,
                                    bias=nm[:], scale=1.0,
                                    accum_out=l0[:, ci:ci + 1])
                            nc.scalar.activation(l0[:, nch:nch + 1],
                                                 sink_sb[:, h:h + 1], ACTF.Exp,
                                                 bias=nm[:], scale=1.0)
                            lsum = pbw.tile([P, 1], F32, tag="lsum")
                            nc.vector.reduce_sum(lsum[:], l0[:, :nch + 1],
                                                 axis=AX.X)
                            linv = pbw.tile([P, 1], F32, tag="linv")
                            nc.vector.reciprocal(linv[:], lsum[:])
                            # transpose p
                            pT_sb = pbw.tile([P, cfg.s], BF16, tag="pT")
                            for g in range((i + 1 + 3) // 4):
                                jn = min(4, (i + 1) - g * 4)
                                tpp = pst.tile([P, 512], BF16, tag="t", bufs=2)
                                for j4 in range(jn):
                                    j = g * 4 + j4
                                    nc.tensor.transpose(
                                        tpp[:, j4 * P:(j4 + 1) * P],
                                        p_sb[:, j * P:(j + 1) * P], ident[:])
                                nc.any.tensor_copy(
                                    pT_sb[:, g * 512:g * 512 + jn * P],
                                    tpp[:, :jn * P])
                            # o = p^T-weighted sum of kv rows
                            o_ps = pso.tile([P, HD], F32, tag="o", bufs=2)
                            for j in range(i + 1):
                                nc.tensor.matmul(o_ps[:],
                                                 pT_sb[:, j * P:(j + 1) * P],
                                                 kv_sb[:, j, :],
                                                 start=(j == 0), stop=(j == i))
                            # normalize + inverse rope + store transposed
                            o_sb = pbw.tile([P, HD], BF16, tag="o_sb")
                            nc.scalar.mul(o_sb[:], o_ps[:], linv[:])
                            _rope_inplace(nc, pbw, o_sb[:],
                                          cos_sb[:, i, :], sin_sb[:, i, :],
                                          True)
                            tpo = pst.tile([P, 512], BF16, tag="t", bufs=2)
                            for c in range(hc):
                                nc.tensor.transpose(
                                    tpo[:, c * P:(c + 1) * P],
                                    o_sb[:, c * P:(c + 1) * P], ident[:])
                            nc.any.tensor_copy(
                                oT_sb[:, h * hc:(h + 1) * hc,
                                      i * P:(i + 1) * P],
                                tpo[:].rearrange("p (c s) -> p c s", c=hc))

                if dbg:
                    nc.sync.dma_start(dbg["ot"], oT_sb[:])

                # ============ stage D: grouped low-rank out proj (wo_a) =====
                with tc.tile_pool(name="ogt", bufs=1) as ogtp:
                    ogT_sb = ogtp.tile([P, cfg.oc, cfg.s], BF16)
                    with tc.tile_pool(name="stD", bufs=1) as pd, \
                         tc.tile_pool(name="psD", bufs=1, space="PSUM") as psd:
                        woa_sb = pd.tile([P, cfg.fc, cfg.olr], BF16)
                        nspd = min(8, cfg.fc)
                        for g in range(nspd):
                            gsz = cfg.fc // nspd
                            nc.sync.dma_start(
                                woa_sb[:, g * gsz:(g + 1) * gsz, :],
                                woa_d[:, g * gsz:(g + 1) * gsz, :])
                        s_chunks = [(a, min(512, cfg.s - a))
                                    for a in range(0, cfg.s, 512)]
                        for m in range(cfg.oc):
                            og_ps = []
                            for n2 in range(len(s_chunks)):
                                og_ps.append(psd.tile([P, 512], F32, tag="og",
                                                      bufs=4, name="og_ps"))
                            for k in range(cfg.fc):
                                for n2, (a, w) in enumerate(s_chunks):
                                    nc.tensor.matmul(
                                        og_ps[n2][:, :w],
                                        woa_sb[:, k, m * P:(m + 1) * P],
                                        oT_sb[:, k, a:a + w],
                                        start=(k == 0), stop=(k == cfg.fc - 1))
                            for n2, (a, w) in enumerate(s_chunks):
                                nc.any.tensor_copy(
                                    ogT_sb[:, m, a:a + w], og_ps[n2][:, :w])

                    if dbg:
                        nc.sync.dma_start(dbg["ogt"], ogT_sb[:])

                    # ============ stage E: final wo_b partial matmul ========
                    with tc.tile_pool(name="stE", bufs=1) as pe, \
                         tc.tile_pool(name="stEw", bufs=4) as pew, \
                         tc.tile_pool(name="psE", bufs=1, space="PSUM") as pse:
                        wob_sb = pe.tile([P, cfg.oc, cfg.outd], BF16)
                        for g in range(cfg.oc):
                            nc.sync.dma_start(wob_sb[:, g:g + 1, :],
                                              wob_d[:, g:g + 1, :])
                        for m in range(sc):
                            out_ps = []
                            for n in range(cfg.nc_out):
                                out_ps.append(pse.tile([P, 512], F32,
                                                       tag="out", bufs=8,
                                                       name="out_ps"))
                            for k in range(cfg.oc):
                                for n in range(cfg.nc_out):
                                    nc.tensor.matmul(
                                        out_ps[n][:],
                                        ogT_sb[:, k, m * P:(m + 1) * P],
                                        wob_sb[:, k, n * 512:(n + 1) * 512],
                                        start=(k == 0), stop=(k == cfg.oc - 1))
                            for n in range(cfg.nc_out):
                                o_out = pew.tile([P, 512], F32, tag="oo")
                                nc.any.tensor_copy(o_out[:], out_ps[n][:])
                                nc.sync.dma_start(
                                    out_d[m, :, n * 512:(n + 1) * 512],
                                    o_out[:])


# ---------------------------------------------------------------------------
# host side
# ---------------------------------------------------------------------------

def _pack_kt(w, n_rows, n_cols):
    """Pack W (given as [n_cols, n_rows] np array) into [128, n_rows/128,
    n_cols] = W.T tiled with the contraction dim on partitions."""
    wt = np.ascontiguousarray(w.T)  # [n_rows, n_cols]
    return np.ascontiguousarray(
        wt.reshape(n_rows // P, P, n_cols).transpose(1, 0, 2))


def prepare_inmaps(inputs, cfg: Cfg):
    bf = NPBF16
    x = np.asarray(inputs["x"], dtype=bf).reshape(cfg.s, cfg.d)
    xt = np.ascontiguousarray(
        x.T.reshape(cfg.dc, P, cfg.sc, P).transpose(2, 1, 0, 3))

    wq_a = np.asarray(inputs["wq_a"], dtype=bf)
    wqa = _pack_kt(wq_a, cfg.d, cfg.qlr)

    wkv = _pack_kt(np.asarray(inputs["wkv"], dtype=bf), cfg.d, HD)

    q_norm_w = np.asarray(inputs["q_norm_w"], dtype=np.float32)
    wq_b = np.asarray(inputs["wq_b"], dtype=bf).astype(np.float32)
    wq_b = (wq_b * q_norm_w[None, :]).astype(bf)  # fold q_norm into wq_b

    kv_norm_w = np.asarray(inputs["kv_norm_w"], dtype=bf)
    kvw = np.ascontiguousarray(np.broadcast_to(kv_norm_w, (P, HD)))

    cos = np.asarray(inputs["cos"], dtype=np.float32)
    sin = np.asarray(inputs["sin"], dtype=np.float32)
    cos_p = np.ascontiguousarray(
        cos.reshape(cfg.sc, P, RD // 2).transpose(1, 0, 2))
    sin_p = np.ascontiguousarray(
        sin.reshape(cfg.sc, P, RD // 2).transpose(1, 0, 2))

    wo_a = np.asarray(inputs["wo_a"], dtype=bf)  # [OG*OLR, F]
    wo_b = np.asarray(inputs["wo_b"], dtype=bf)  # [D, OG*OLR]
    sink = np.asarray(inputs["attn_sink"], dtype=np.float32)

    xt_tiles = xt  # [sc, P, dc, P]
    in_maps = []
    for c in range(NCORES):
        h0 = c * cfg.hpc
        wqb_c = wq_b[h0 * HD:(h0 + cfg.hpc) * HD, :]  # [hpc*HD, qlr]
        woa_c = wo_a[c * cfg.olr:(c + 1) * cfg.olr, :]  # [olr, F]
        wob_c = wo_b[:, c * cfg.olr:(c + 1) * cfg.olr]  # [outd, olr]
        sink_c = sink[h0:h0 + cfg.hpc]
        in_maps.append({
            "xtm": np.ascontiguousarray(xt_tiles[c]),
            "cosm": np.ascontiguousarray(cos_p[:, c, :]),
            "sinm": np.ascontiguousarray(sin_p[:, c, :]),
            "wqa": wqa,
            "wkv": wkv,
            "wqb": _pack_kt(wqb_c, cfg.qlr, cfg.hpc * HD),
            "woa": _pack_kt(woa_c, cfg.f, cfg.olr),
            "wob": _pack_kt(wob_c, cfg.olr, cfg.outd),
            "coss": cos_p,
            "sins": sin_p,
            "kvw": kvw,
            "sink": np.ascontiguousarray(np.broadcast_to(sink_c, (P, cfg.hpc))),
            "nsink": np.ascontiguousarray(
                np.broadcast_to(-sink_c, (P, cfg.hpc))),
        })
    return in_maps


_CACHE = {}


def _get_program():
    if "nc" not in _CACHE:
        _CACHE["nc"] = build_program(Cfg())
    return _CACHE["nc"]


def run(inputs, trace=False):
    """Returns (output [1,S,D] bf16, BassKernelResults)."""
    cfg = Cfg()
    nc = _get_program()
    in_maps = prepare_inmaps(inputs, cfg)
    res = run_bass_kernel_spmd(nc, in_maps, core_ids=list(range(NCORES)),
                               trace=trace)
    acc = np.zeros((cfg.s, cfg.outd), np.float32)
    for r in res.results:
        acc += r["out"].reshape(cfg.s, cfg.outd)
    out = acc.astype(NPBF16).reshape(1, cfg.s, cfg.outd)
    return out, res


def kernel(**inputs) -> np.ndarray:
    out, _ = run(inputs)
    return out


# revision 16
# speedup vs baseline: 1.6854x; 1.6854x over previous
"""MLA-style attention (shared latent KV head, attention sink, partial RoPE,
low-rank Q and grouped low-rank output projection) on 8 TRN2 NeuronCores.

Sharding: 64 query heads split 8 per core (tensor parallel on wq_b rows /
wo_a groups); latent KV path replicated; final wo_b matmul computed as
per-core partial products (each core owns one OLR group / one 1024-col slice
of wo_b) summed on the host.

All weights / activations are pre-laid-out on the host into the exact
[partition, ...] tile shapes the kernel wants, so every DMA is a contiguous
copy and the device never transposes anything except through the PE array
(qr -> qrT, q -> qT, p -> pT, o -> oT, kv -> kvT).
"""

import numpy as np
import ml_dtypes

import concourse.bass as bass
import concourse.mybir as mybir
import concourse.tile as tile
from concourse import bacc
from concourse.bass_utils import run_bass_kernel_spmd
from concourse.masks import make_identity, make_causal_mask

BF16 = mybir.dt.bfloat16
F32 = mybir.dt.float32
AX = mybir.AxisListType
ALU = mybir.AluOpType
ACTF = mybir.ActivationFunctionType

NPBF16 = ml_dtypes.bfloat16

# problem dims (hardcoded; kernel.py must be self-contained)
D, NH, HD, RD, QLR, OLR, OG = 4096, 64, 512, 64, 1024, 1024, 8
S = 1024
NCORES = 8
HPC = NH // NCORES  # query heads per core
EPS = 1e-6
P = 128


class Cfg:
    """Dimensions, parameterized so a shrunken config can run in CoreSim."""

    def __init__(self, s=S, d=D, qlr=QLR, hpc=HPC, olr=OLR, outd=D):
        assert s % P == 0 and d % P == 0 and qlr % 512 == 0 and olr % 512 == 0
        assert outd % 512 == 0
        self.s, self.d, self.qlr, self.hpc, self.olr, self.outd = (
            s, d, qlr, hpc, olr, outd)
        self.sc = s // P        # seq tiles
        self.dc = d // P        # model-dim chunks (contraction for qr/kv)
        self.qc = qlr // P      # q_lora chunks
        self.hc = HD // P       # head-dim chunks (4)
        self.f = hpc * HD       # per-core attention output feature dim
        self.fc = self.f // P   # feature chunks for wo_a contraction
        self.oc = olr // P      # olr chunks (contraction for wo_b)
        self.nc_out = outd // 512  # output D chunks


def _rope_inplace(nc, pool, dst, cos_ap, sin_ap, inverse):
    """Partial RoPE on dst[:, HD-RD:HD] in place. dst is [128, HD] bf16,
    cos/sin are [128, RD//2] f32 for this seq tile."""
    tail = dst[:, HD - RD:HD].rearrange("p (a two) -> p a two", two=2)
    x1 = tail[:, :, 0]
    x2 = tail[:, :, 1]
    t1 = pool.tile([P, RD // 2], F32, tag="rope1")
    t2 = pool.tile([P, RD // 2], F32, tag="rope2")
    t3 = pool.tile([P, RD // 2], F32, tag="rope3")
    t4 = pool.tile([P, RD // 2], F32, tag="rope4")
    nc.vector.tensor_mul(t1[:], x1, cos_ap)
    nc.vector.tensor_mul(t2[:], x2, sin_ap)
    nc.vector.tensor_mul(t3[:], x1, sin_ap)
    nc.vector.tensor_mul(t4[:], x2, cos_ap)
    if not inverse:
        # o1 = x1 c - x2 s ; o2 = x1 s + x2 c
        nc.vector.tensor_sub(x1, t1[:], t2[:])
        nc.vector.tensor_add(x2, t3[:], t4[:])
    else:
        # o1 = x1 c + x2 s ; o2 = x2 c - x1 s
        nc.vector.tensor_add(x1, t1[:], t2[:])
        nc.vector.tensor_sub(x2, t4[:], t3[:])


def build_program(cfg: Cfg, debug=False, reps=1, shard_a=None):
    """Trace + schedule + compile the per-core program. Returns nc.
    reps>1 repeats the whole body (for steady-state timing)."""
    nc = bacc.Bacc("TRN2", debug=False, num_devices=NCORES)

    # ---- DRAM I/O (host supplies pre-tiled layouts) ----
    if shard_a is None:
        shard_a = (cfg.sc == NCORES)
    if shard_a:
        xt_d = nc.dram_tensor("xtm", [P, cfg.dc, P], BF16,
                              kind="ExternalInput").ap()
        cosm_d = nc.dram_tensor("cosm", [P, RD // 2], F32,
                                kind="ExternalInput").ap()
        sinm_d = nc.dram_tensor("sinm", [P, RD // 2], F32,
                                kind="ExternalInput").ap()
    else:
        xt_d = nc.dram_tensor("xt", [cfg.sc, P, cfg.dc, P], BF16,
                              kind="ExternalInput").ap()
        cosm_d = sinm_d = None
    wqa_d = nc.dram_tensor("wqa", [P, cfg.dc, cfg.qlr], BF16,
                           kind="ExternalInput").ap()
    wkv_d = nc.dram_tensor("wkv", [P, cfg.dc, HD], BF16,
                           kind="ExternalInput").ap()
    wqb_d = nc.dram_tensor("wqb", [P, cfg.qc, cfg.hpc * HD], BF16,
                           kind="ExternalInput").ap()
    woa_d = nc.dram_tensor("woa", [P, cfg.fc, cfg.olr], BF16,
                           kind="ExternalInput").ap()
    wob_d = nc.dram_tensor("wob", [P, cfg.oc, cfg.outd], BF16,
                           kind="ExternalInput").ap()
    cos_d = nc.dram_tensor("coss", [P, cfg.sc, RD // 2], F32,
                           kind="ExternalInput").ap()
    sin_d = nc.dram_tensor("sins", [P, cfg.sc, RD // 2], F32,
                           kind="ExternalInput").ap()
    kvw_d = nc.dram_tensor("kvw", [P, HD], BF16, kind="ExternalInput").ap()
    sink_d = nc.dram_tensor("sink", [P, cfg.hpc], F32,
                            kind="ExternalInput").ap()
    nsink_d = nc.dram_tensor("nsink", [P, cfg.hpc], F32,
                             kind="ExternalInput").ap()
    out_d = nc.dram_tensor("out", [cfg.sc, P, cfg.outd], F32,
                           kind="ExternalOutput").ap()
    dbg = {}
    if debug:
        dbg["qrt"] = nc.dram_tensor("dbg_qrt", [P, cfg.qc, cfg.s], BF16,
                                    kind="ExternalOutput").ap()
        dbg["kv"] = nc.dram_tensor("dbg_kv", [P, cfg.sc, HD], BF16,
                                   kind="ExternalOutput").ap()
        dbg["qt0"] = nc.dram_tensor("dbg_qt0", [P, cfg.hc, cfg.s], BF16,
                                    kind="ExternalOutput").ap()
        dbg["ogt"] = nc.dram_tensor("dbg_ogt", [P, cfg.oc, cfg.s], BF16,
                                    kind="ExternalOutput").ap()

    with tile.TileContext(nc) as tc:
        for _ in range(reps):
            _body(nc, tc, cfg, xt_d, wqa_d, wkv_d, wqb_d, woa_d, wob_d,
                  cos_d, sin_d, kvw_d, sink_d, nsink_d, out_d, dbg,
                  shard_a=shard_a, cosm_d=cosm_d, sinm_d=sinm_d)

    nc.compile()
    return nc


def _body(nc, tc, cfg, xt_d, wqa_d, wkv_d, wqb_d, woa_d, wob_d,
          cos_d, sin_d, kvw_d, sink_d, nsink_d, out_d, dbg=None,
          shard_a=False, cosm_d=None, sinm_d=None):
    sc, dc, qc, hc = cfg.sc, cfg.dc, cfg.qc, cfg.hc

    with tc.tile_pool(name="persist", bufs=1) as pp:
        ident = pp.tile([P, P], BF16)
        make_identity(nc, ident[:])
        cmask = pp.tile([P, P], F32)
        make_causal_mask(nc, cmask[:], mask_val=-1e10)
        kvw_sb = pp.tile([P, HD], BF16)
        nc.sync.dma_start(kvw_sb[:], kvw_d)
        sink_sb = pp.tile([P, cfg.hpc], F32)
        nc.sync.dma_start(sink_sb[:], sink_d)
        nsink_sb = pp.tile([P, cfg.hpc], F32)
        nc.sync.dma_start(nsink_sb[:], nsink_d)
        cos_sb = pp.tile([P, sc, RD // 2], F32)
        nc.sync.dma_start(cos_sb[:], cos_d)
        sin_sb = pp.tile([P, sc, RD // 2], F32)
        nc.sync.dma_start(sin_sb[:], sin_d)
        if shard_a:
            cosm_sb = pp.tile([P, RD // 2], F32)
            nc.sync.dma_start(cosm_sb[:], cosm_d)
            sinm_sb = pp.tile([P, RD // 2], F32)
            nc.sync.dma_start(sinm_sb[:], sinm_d)
        kv_sb = pp.tile([P, sc, HD], BF16)      # latent KV, [s-in-tile, tile, hd]
        kvT_sb = pp.tile([P, hc, cfg.s], BF16)  # latent KV transposed
        eps_sb = pp.tile([P, 2], F32)           # [:,0]=EPS, [:,1]=-ln(HD)/2
        nc.gpsimd.memset(eps_sb[:, 0:1], float(EPS))
        nc.gpsimd.memset(eps_sb[:, 1:2], float(-0.5 * np.log(HD)))

        with tc.tile_pool(name="qrt", bufs=1) as qrtp:
            qrT_sb = qrtp.tile([P, qc, cfg.s], BF16)

            # ================= stage A: qr + kv =================
            with tc.tile_pool(name="stA", bufs=1) as pa, \
                 tc.tile_pool(name="stAw", bufs=2) as paw, \
                 tc.tile_pool(name="psA", bufs=1, space="PSUM") as psa:
                # weights, chunked so compute can start early
                wqa_sb = pa.tile([P, dc, cfg.qlr], BF16)
                nsp = min(8, dc)
                for g in range(nsp):
                    gsz = dc // nsp
                    nc.sync.dma_start(wqa_sb[:, g * gsz:(g + 1) * gsz, :],
                                      wqa_d[:, g * gsz:(g + 1) * gsz, :])
                wkv_sb = pa.tile([P, dc, HD], BF16)
                nsp = min(4, dc)
                for g in range(nsp):
                    gsz = dc // nsp
                    nc.sync.dma_start(wkv_sb[:, g * gsz:(g + 1) * gsz, :],
                                      wkv_d[:, g * gsz:(g + 1) * gsz, :])

                for i in range([sc, 1][shard_a]):
                    xt_i = paw.tile([P, dc, P], BF16, tag="xt")
                    nc.sync.dma_start(xt_i[:], xt_d if shard_a else xt_d[i])
                    qr_ps = psa.tile([P, cfg.qlr], F32, tag="qr", bufs=2)
                    kv_ps = psa.tile([P, HD], F32, tag="kv", bufs=2)
                    for k in range(dc):
                        st, sp = k == 0, k == dc - 1
                        for n2 in range(cfg.qlr // 512):
                            nc.tensor.matmul(
                                qr_ps[:, n2 * 512:(n2 + 1) * 512],
                                xt_i[:, k, :],
                                wqa_sb[:, k, n2 * 512:(n2 + 1) * 512],
                                start=st, stop=sp)
                        nc.tensor.matmul(kv_ps[:], xt_i[:, k, :],
                                         wkv_sb[:, k, :], start=st, stop=sp)

                    # --- qr epilogue: cast, rmsnorm, transpose ---
                    qr_sb = paw.tile([P, cfg.qlr], BF16, tag="qr_sb")
                    nc.any.tensor_copy(qr_sb[:], qr_ps[:])
                    sq = paw.tile([P, cfg.qlr], F32, tag="sq")
                    ssq = paw.tile([P, 1], F32, tag="ssq")
                    nc.scalar.activation(sq[:], qr_sb[:], ACTF.Square,
                                         accum_out=ssq[:])
                    rt = paw.tile([P, 1], F32, tag="rt")
                    nc.scalar.activation(rt[:], ssq[:], ACTF.Ln,
                                         bias=eps_sb[:, 0:1],
                                         scale=1.0 / cfg.qlr)
                    rinv = paw.tile([P, 1], F32, tag="rinv")
                    nc.scalar.activation(rinv[:], rt[:], ACTF.Exp, scale=-0.5)
                    qrn = paw.tile([P, cfg.qlr], BF16, tag="qrn")
                    nc.scalar.mul(qrn[:], qr_sb[:], rinv[:])
                    if shard_a:
                        qrT_loc = paw.tile([P, qc, P], BF16, tag="qrT_loc",
                                           bufs=1)
                    for g in range((qc + 3) // 4):
                        jn = min(4, qc - g * 4)
                        tp = psa.tile([P, 512], BF16, tag="t", bufs=2)
                        for c4 in range(jn):
                            c = g * 4 + c4
                            nc.tensor.transpose(
                                tp[:, c4 * P:(c4 + 1) * P],
                                qrn[:, c * P:(c + 1) * P], ident[:])
                        dst = (qrT_loc[:, g * 4:g * 4 + jn, :] if shard_a else
                               qrT_sb[:, g * 4:g * 4 + jn, i * P:(i + 1) * P])
                        nc.any.tensor_copy(
                            dst,
                            tp[:, :jn * P].rearrange("p (c s) -> p c s", c=jn))

                    # --- kv epilogue: cast, rmsnorm, weight, rope, transpose ---
                    kvt = paw.tile([P, HD], BF16, tag="kvt")
                    nc.any.tensor_copy(kvt[:], kv_ps[:])
                    sqk = paw.tile([P, HD], F32, tag="sqk")
                    ssqk = paw.tile([P, 1], F32, tag="ssqk")
                    nc.scalar.activation(sqk[:], kvt[:], ACTF.Square,
                                         accum_out=ssqk[:])
                    rtk = paw.tile([P, 1], F32, tag="rtk")
                    nc.scalar.activation(rtk[:], ssqk[:], ACTF.Ln,
                                         bias=eps_sb[:, 0:1], scale=1.0 / HD)
                    rinvk = paw.tile([P, 1], F32, tag="rinvk")
                    nc.scalar.activation(rinvk[:], rtk[:], ACTF.Exp,
                                         scale=-0.5)
                    if shard_a:
                        kv_dst = paw.tile([P, HD], BF16, tag="kv_loc", bufs=1)
                        kv_dst = kv_dst[:]
                        cos_i, sin_i = cosm_sb[:], sinm_sb[:]
                    else:
                        kv_dst = kv_sb[:, i, :]
                        cos_i, sin_i = cos_sb[:, i, :], sin_sb[:, i, :]
                    nc.scalar.mul(kv_dst, kvt[:], rinvk[:])
                    nc.vector.tensor_mul(kv_dst, kv_dst, kvw_sb[:])
                    _rope_inplace(nc, paw, kv_dst, cos_i, sin_i, False)
                    tpk = psa.tile([P, 512], BF16, tag="t", bufs=2)
                    for c in range(hc):
                        nc.tensor.transpose(tpk[:, c * P:(c + 1) * P],
                                            kv_dst[:, c * P:(c + 1) * P],
                                            ident[:])
                    if shard_a:
                        kvT_loc = paw.tile([P, hc, P], BF16, tag="kvT_loc",
                                           bufs=1)
                        nc.any.tensor_copy(
                            kvT_loc[:],
                            tpk[:].rearrange("p (c s) -> p c s", c=hc))
                        # pack local results into DRAM and all-gather
                        gw = qc * P + HD + hc * P      # 2048 for full cfg
                        with tc.tile_pool(name="ccdram", bufs=1,
                                          space="DRAM") as ccd:
                            gin = ccd.tile([P, gw], BF16)
                            gout = ccd.tile([NCORES, P, gw], BF16)
                            nc.sync.dma_start(gin[:, 0:qc * P],
                                              qrT_loc[:].rearrange(
                                                  "p c s -> p (c s)"))
                            nc.sync.dma_start(
                                gin[:, qc * P:qc * P + HD], kv_dst)
                            nc.sync.dma_start(gin[:, qc * P + HD:gw],
                                              kvT_loc[:].rearrange(
                                                  "p c s -> p (c s)"))
                            nc.gpsimd.collective_compute(
                                "AllGather", ALU.bypass,
                                replica_groups=[list(range(NCORES))],
                                ins=[gin[:]], outs=[gout[:]])
                            for j in range(NCORES):
                                nc.sync.dma_start(
                                    qrT_sb[:, :, j * P:(j + 1) * P],
                                    gout[j, :, 0:qc * P].rearrange(
                                        "p (c s) -> p c s", c=qc))
                                nc.sync.dma_start(
                                    kv_sb[:, j, :],
                                    gout[j, :, qc * P:qc * P + HD])
                                nc.sync.dma_start(
                                    kvT_sb[:, :, j * P:(j + 1) * P],
                                    gout[j, :, qc * P + HD:gw].rearrange(
                                        "p (c s) -> p c s", c=hc))
                    else:
                        nc.any.tensor_copy(
                            kvT_sb[:, :, i * P:(i + 1) * P],
                            tpk[:].rearrange("p (c s) -> p c s", c=hc))

            if dbg:
                nc.sync.dma_start(dbg["qrt"], qrT_sb[:])
                nc.sync.dma_start(dbg["kv"], kv_sb[:])

            # ====== stage BC: per-head q proj + attention + wo_a partial ======
            s_chunks = [(a, min(512, cfg.s - a))
                        for a in range(0, cfg.s, 512)]
            with tc.tile_pool(name="og", bufs=1) as ogp:
                og_acc = ogp.tile([P, cfg.oc, cfg.s], F32)

                with tc.tile_pool(name="stBC", bufs=1) as pb, \
                     tc.tile_pool(name="stBCw", bufs=2) as pbw, \
                     tc.tile_pool(name="psQ", bufs=1, space="PSUM") as psq, \
                     tc.tile_pool(name="psS", bufs=1, space="PSUM") as pss, \
                     tc.tile_pool(name="psT", bufs=1, space="PSUM") as pst, \
                     tc.tile_pool(name="psO", bufs=1, space="PSUM") as pso, \
                     tc.tile_pool(name="psD", bufs=1, space="PSUM") as psd:
                    woa_sb = pb.tile([P, cfg.fc, cfg.olr], BF16)
                    nspd = min(8, cfg.fc)
                    for g in range(nspd):
                        gsz = cfg.fc // nspd
                        nc.sync.dma_start(
                            woa_sb[:, g * gsz:(g + 1) * gsz, :],
                            woa_d[:, g * gsz:(g + 1) * gsz, :])

                    for h in range(cfg.hpc):
                        wqb_h = pbw.tile([P, qc, HD], BF16, tag="wqb_h")
                        nc.sync.dma_start(wqb_h[:],
                                          wqb_d[:, :, h * HD:(h + 1) * HD])
                        qT_sb = pbw.tile([P, hc, cfg.s], BF16, tag="qT")
                        # ---- q projection + per-head RMS norm + rope ----
                        q8 = pbw.tile([P, sc, HD], BF16, tag="q8", bufs=1)
                        ssq8 = pbw.tile([P, sc], F32, tag="ssq8")
                        for i in range(sc):
                            q_ps = psq.tile([P, HD], F32, tag="q", bufs=1)
                            for c in range(qc):
                                nc.tensor.matmul(
                                    q_ps[:],
                                    qrT_sb[:, c, i * P:(i + 1) * P],
                                    wqb_h[:, c, :],
                                    start=(c == 0), stop=(c == qc - 1))
                            nc.any.tensor_copy(q8[:, i, :], q_ps[:])
                            sqq = pbw.tile([P, HD], F32, tag="sqq")
                            nc.scalar.activation(sqq[:], q8[:, i, :],
                                                 ACTF.Square,
                                                 accum_out=ssq8[:, i:i + 1])
                        # rsqrt(ms+eps)/sqrt(HD) = exp(-0.5*ln(ssq/HD+eps)
                        #                              - 0.5*ln(HD))
                        rt8 = pbw.tile([P, sc], F32, tag="rt8")
                        nc.scalar.activation(rt8[:], ssq8[:], ACTF.Ln,
                                             bias=eps_sb[:, 0:1],
                                             scale=1.0 / HD)
                        rinv8 = pbw.tile([P, sc], F32, tag="rinv8")
                        nc.scalar.activation(rinv8[:], rt8[:], ACTF.Exp,
                                             scale=-0.5,
                                             bias=eps_sb[:, 1:2])
                        for i in range(sc):
                            nc.scalar.mul(q8[:, i, :], q8[:, i, :],
                                          rinv8[:, i:i + 1])
                            _rope_inplace(nc, pbw, q8[:, i, :],
                                          cos_sb[:, i, :], sin_sb[:, i, :],
                                          False)
                            tpq = pst.tile([P, 512], BF16, tag="t", bufs=2)
                            for c in range(hc):
                                nc.tensor.transpose(
                                    tpq[:, c * P:(c + 1) * P],
                                    q8[:, i, c * P:(c + 1) * P], ident[:])
                            nc.any.tensor_copy(
                                qT_sb[:, :, i * P:(i + 1) * P],
                                tpq[:].rearrange("p (c s) -> p c s", c=hc))

                        if dbg and h == 0:
                            nc.sync.dma_start(dbg["qt0"], qT_sb[:])

                        # ---- attention for head h ----
                        oT_h = pbw.tile([P, hc, cfg.s], BF16, tag="oT_h")
                        for i in range(sc):
                            w_all = (i + 1) * P
                            nch = (w_all + 511) // 512
                            s_ps = []
                            for ci in range(nch):
                                wci = min(512, w_all - ci * 512)
                                s_ps.append((pss.tile([P, 512], F32, tag="s",
                                                      bufs=2, name="s_ps"),
                                             wci))
                            for k in range(hc):
                                for ci in range(nch):
                                    tile_ps, wci = s_ps[ci]
                                    nc.tensor.matmul(
                                        tile_ps[:, :wci],
                                        qT_sb[:, k, i * P:(i + 1) * P],
                                        kvT_sb[:, k, ci * 512:ci * 512 + wci],
                                        start=(k == 0), stop=(k == hc - 1))
                            # causal mask on the diagonal block
                            dps, dw = s_ps[-1]
                            dcol = (w_all - P) - (nch - 1) * 512
                            nc.vector.tensor_add(dps[:, dcol:dcol + P],
                                                 dps[:, dcol:dcol + P],
                                                 cmask[:])
                            # negated row max (incl. sink)
                            nmt = pbw.tile([P, 3], F32, tag="nmt")
                            for ci in range(nch):
                                tile_ps, wci = s_ps[ci]
                                nc.vector.reduce_max(nmt[:, ci:ci + 1],
                                                     tile_ps[:, :wci],
                                                     axis=AX.X, negate=True)
                            nm = pbw.tile([P, 1], F32, tag="nm")
                            if nch == 1:
                                nc.vector.tensor_tensor(
                                    nm[:], nmt[:, 0:1], nsink_sb[:, h:h + 1],
                                    op=ALU.min)
                            else:
                                nc.vector.tensor_tensor(
                                    nm[:], nmt[:, 0:1], nmt[:, 1:2],
                                    op=ALU.min)
                                nc.vector.tensor_tensor(
                                    nm[:], nm[:], nsink_sb[:, h:h + 1],
                                    op=ALU.min)
                            # exp + row sums
                            p_sb = pbw.tile([P, cfg.s], BF16, tag="p")
                            l0 = pbw.tile([P, 4], F32, tag="l0")
                            for ci in range(nch):
                                tile_ps, wci = s_ps[ci]
                                nc.scalar.activation(
                                    p_sb[:, ci * 512:ci * 512 + wci],
                                    tile_ps[:, :wci], ACTF.Exp,
                                    bias=nm[:], scale=1.0,
                                    accum_out=l0[:, ci:ci + 1])
                            nc.scalar.activation(l0[:, nch:nch + 1],
                                                 sink_sb[:, h:h + 1], ACTF.Exp,
                                                 bias=nm[:], scale=1.0)
                            lsum = pbw.tile([P, 1], F32, tag="lsum")
                            nc.vector.reduce_sum(lsum[:], l0[:, :nch + 1],
                                                 axis=AX.X)
                            linv = pbw.tile([P, 1], F32, tag="linv")
                            nc.vector.reciprocal(linv[:], lsum[:])
                            # transpose p
                            pT_sb = pbw.tile([P, cfg.s], BF16, tag="pT")
                            for g in range((i + 1 + 3) // 4):
                                jn = min(4, (i + 1) - g * 4)
                                tpp = pst.tile([P, 512], BF16, tag="t", bufs=2)
                                for j4 in range(jn):
                                    j = g * 4 + j4
                                    nc.tensor.transpose(
                                        tpp[:, j4 * P:(j4 + 1) * P],
                                        p_sb[:, j * P:(j + 1) * P], ident[:])
                                nc.any.tensor_copy(
                                    pT_sb[:, g * 512:g * 512 + jn * P],
                                    tpp[:, :jn * P])
                            # o = p^T-weighted sum of kv rows
                            o_ps = pso.tile([P, HD], F32, tag="o", bufs=1)
                            for j in range(i + 1):
                                nc.tensor.matmul(o_ps[:],
                                                 pT_sb[:, j * P:(j + 1) * P],
                                                 kv_sb[:, j, :],
                                                 start=(j == 0), stop=(j == i))
                            # normalize + inverse rope + store transposed
                            o_sb = pbw.tile([P, HD], BF16, tag="o_sb")
                            nc.scalar.mul(o_sb[:], o_ps[:], linv[:])
                            _rope_inplace(nc, pbw, o_sb[:],
                                          cos_sb[:, i, :], sin_sb[:, i, :],
                                          True)
                            tpo = pst.tile([P, 512], BF16, tag="t", bufs=2)
                            for c in range(hc):
                                nc.tensor.transpose(
                                    tpo[:, c * P:(c + 1) * P],
                                    o_sb[:, c * P:(c + 1) * P], ident[:])
                            nc.any.tensor_copy(
                                oT_h[:, :, i * P:(i + 1) * P],
                                tpo[:].rearrange("p (c s) -> p c s", c=hc))

                        # ---- wo_a partial for this head, into f32 og_acc ----
                        for m in range(cfg.oc):
                            d_ps = []
                            for n2 in range(len(s_chunks)):
                                d_ps.append(psd.tile([P, 512], F32, tag="d",
                                                     bufs=2, name="d_ps"))
                            for kk in range(hc):
                                k = h * hc + kk
                                for n2, (a, w) in enumerate(s_chunks):
                                    nc.tensor.matmul(
                                        d_ps[n2][:, :w],
                                        woa_sb[:, k, m * P:(m + 1) * P],
                                        oT_h[:, kk, a:a + w],
                                        start=(kk == 0), stop=(kk == hc - 1))
                            for n2, (a, w) in enumerate(s_chunks):
                                if h == 0:
                                    nc.vector.tensor_copy(
                                        og_acc[:, m, a:a + w], d_ps[n2][:, :w])
                                else:
                                    nc.vector.tensor_add(
                                        og_acc[:, m, a:a + w],
                                        og_acc[:, m, a:a + w], d_ps[n2][:, :w])

                # ============ stage E: final wo_b partial matmul ========
                with tc.tile_pool(name="stE", bufs=1) as pe, \
                     tc.tile_pool(name="stEw", bufs=4) as pew, \
                     tc.tile_pool(name="psE", bufs=1, space="PSUM") as pse:
                    wob_sb = pe.tile([P, cfg.oc, cfg.outd], BF16)
                    for g in range(cfg.oc):
                        nc.sync.dma_start(wob_sb[:, g:g + 1, :],
                                          wob_d[:, g:g + 1, :])
                    ogT_sb = pe.tile([P, cfg.oc, cfg.s], BF16)
                    for m in range(cfg.oc):
                        for a, w in s_chunks:
                            nc.any.tensor_copy(ogT_sb[:, m, a:a + w],
                                               og_acc[:, m, a:a + w])
                    if dbg:
                        nc.sync.dma_start(dbg["ogt"], ogT_sb[:])
                    for m in range(sc):
                        out_ps = []
                        for n in range(cfg.nc_out):
                            out_ps.append(pse.tile([P, 512], F32,
                                                   tag="out", bufs=8,
                                                   name="out_ps"))
                        for k in range(cfg.oc):
                            for n in range(cfg.nc_out):
                                nc.tensor.matmul(
                                    out_ps[n][:],
                                    ogT_sb[:, k, m * P:(m + 1) * P],
                                    wob_sb[:, k, n * 512:(n + 1) * 512],
                                    start=(k == 0), stop=(k == cfg.oc - 1))
                        for n in range(cfg.nc_out):
                            o_out = pew.tile([P, 512], F32, tag="oo")
                            nc.any.tensor_copy(o_out[:], out_ps[n][:])
                            nc.sync.dma_start(
                                out_d[m, :, n * 512:(n + 1) * 512],
                                o_out[:])


# ---------------------------------------------------------------------------
# host side
# ---------------------------------------------------------------------------

def _pack_kt(w, n_rows, n_cols):
    """Pack W (given as [n_cols, n_rows] np array) into [128, n_rows/128,
    n_cols] = W.T tiled with the contraction dim on partitions."""
    wt = np.ascontiguousarray(w.T)  # [n_rows, n_cols]
    return np.ascontiguousarray(
        wt.reshape(n_rows // P, P, n_cols).transpose(1, 0, 2))


def prepare_inmaps(inputs, cfg: Cfg, shard_a=True):
    bf = NPBF16
    x = np.asarray(inputs["x"], dtype=bf).reshape(cfg.s, cfg.d)
    xt = np.ascontiguousarray(
        x.T.reshape(cfg.dc, P, cfg.sc, P).transpose(2, 1, 0, 3))

    wq_a = np.asarray(inputs["wq_a"], dtype=bf)
    wqa = _pack_kt(wq_a, cfg.d, cfg.qlr)

    wkv = _pack_kt(np.asarray(inputs["wkv"], dtype=bf), cfg.d, HD)

    q_norm_w = np.asarray(inputs["q_norm_w"], dtype=np.float32)
    wq_b = np.asarray(inputs["wq_b"], dtype=bf).astype(np.float32)
    wq_b = (wq_b * q_norm_w[None, :]).astype(bf)  # fold q_norm into wq_b

    kv_norm_w = np.asarray(inputs["kv_norm_w"], dtype=bf)
    kvw = np.ascontiguousarray(np.broadcast_to(kv_norm_w, (P, HD)))

    cos = np.asarray(inputs["cos"], dtype=np.float32)
    sin = np.asarray(inputs["sin"], dtype=np.float32)
    cos_p = np.ascontiguousarray(
        cos.reshape(cfg.sc, P, RD // 2).transpose(1, 0, 2))
    sin_p = np.ascontiguousarray(
        sin.reshape(cfg.sc, P, RD // 2).transpose(1, 0, 2))

    wo_a = np.asarray(inputs["wo_a"], dtype=bf)  # [OG*OLR, F]
    wo_b = np.asarray(inputs["wo_b"], dtype=bf)  # [D, OG*OLR]
    sink = np.asarray(inputs["attn_sink"], dtype=np.float32)

    xt_tiles = xt  # [sc, P, dc, P]
    in_maps = []
    for c in range(NCORES):
        h0 = c * cfg.hpc
        wqb_c = wq_b[h0 * HD:(h0 + cfg.hpc) * HD, :]  # [hpc*HD, qlr]
        woa_c = wo_a[c * cfg.olr:(c + 1) * cfg.olr, :]  # [olr, F]
        wob_c = wo_b[:, c * cfg.olr:(c + 1) * cfg.olr]  # [outd, olr]
        sink_c = sink[h0:h0 + cfg.hpc]
        core_specific = (
            {"xtm": np.ascontiguousarray(xt_tiles[c]),
             "cosm": np.ascontiguousarray(cos_p[:, c, :]),
             "sinm": np.ascontiguousarray(sin_p[:, c, :])}
            if shard_a else {"xt": xt_tiles})
        in_maps.append({
            **core_specific,
            "wqa": wqa,
            "wkv": wkv,
            "wqb": _pack_kt(wqb_c, cfg.qlr, cfg.hpc * HD),
            "woa": _pack_kt(woa_c, cfg.f, cfg.olr),
            "wob": _pack_kt(wob_c, cfg.olr, cfg.outd),
            "coss": cos_p,
            "sins": sin_p,
            "kvw": kvw,
            "sink": np.ascontiguousarray(np.broadcast_to(sink_c, (P, cfg.hpc))),
            "nsink": np.ascontiguousarray(
                np.broadcast_to(-sink_c, (P, cfg.hpc))),
        })
    return in_maps


_CACHE = {}


def _get_program():
    if "nc" not in _CACHE:
        _CACHE["nc"] = build_program(Cfg())
    return _CACHE["nc"]


def run(inputs, trace=False):
    """Returns (output [1,S,D] bf16, BassKernelResults)."""
    cfg = Cfg()
    nc = _get_program()
    in_maps = prepare_inmaps(inputs, cfg)
    res = run_bass_kernel_spmd(nc, in_maps, core_ids=list(range(NCORES)),
                               trace=trace)
    acc = np.zeros((cfg.s, cfg.outd), np.float32)
    for r in res.results:
        acc += r["out"].reshape(cfg.s, cfg.outd)
    out = acc.astype(NPBF16).reshape(1, cfg.s, cfg.outd)
    return out, res


def kernel(**inputs) -> np.ndarray:
    out, _ = run(inputs)
    return out
